# revision 3
# baseline (speedup 1.0000x reference)
"""Causal MHA kernel for TRN2 — v2.

Sharding: 8 cores = 2 batch-groups x 4 head-groups. Core (bg, hg) computes
batches {2bg, 2bg+1} for heads {4hg..4hg+3}: q/k/v projection, causal
attention, and the output-projection partial product over its 512 input
channels. Host sums the 4 head-group partials per batch (the tensor-parallel
all-reduce), 4x less output traffic + host work than 8-way head sharding.

Datapath is fp16 end-to-end (PE runs fp16 at 1 cycle/row with no free-dim
constraint, DVE gets 4x on fp16 SBUF operands, DMA bytes halve); PSUM stays
fp32. For N(0,1)-scaled data fp16 (10 mantissa bits) is ~8x more precise
than bf16.

Attention is triangle-trimmed: for the diagonal 512x512 block, k-tile j only
computes/exps/accumulates the q-range [128j, 512), and a single 128x128
lower-tri mask handles the partial tile. Softmax denominator: DVE
accumulates exp tiles (fp16, 4x mode), one all-ones matmul per (c,h)
reduces across partitions and broadcasts.
"""
import numpy as np
import ml_dtypes

import concourse.bass as bass
import concourse.mybir as mybir
from concourse.alu_op_type import AluOpType
import concourse.tile as tile
from concourse import bacc
from concourse.bass_utils import run_bass_kernel_spmd

B, S, D = 4, 2048, 2048
H, DK = 16, 128
NCORES = 8
BPC = 2                    # batches per core
HPC = 4                    # heads per core
CD = HPC * DK              # 512 out-channels per core per projection
SC = 512                   # q/score chunk
NSC = S // SC              # 4
NST = SC // 128            # 4 s-tiles per chunk
NDC = D // 128             # 16 d-chunks
F16 = mybir.dt.float16
F32 = mybir.dt.float32
SCALE = 1.0 / np.sqrt(DK)

import os as _os
def _knob(name, default):
    return int(_os.environ.get(name, str(default)))
XS_BUFS = _knob("XS_BUFS", 2)
QT_BUFS = _knob("QT_BUFS", 4)
AT_BUFS = _knob("AT_BUFS", 3)
PT_BUFS = _knob("PT_BUFS", 8)
DEN_BUFS = _knob("DEN_BUFS", 1)
RBC_BUFS = _knob("RBC_BUFS", 1)
OSB_BUFS = _knob("OSB_BUFS", 2)
PS_QKV = _knob("PS_QKV", 2)
PS_S = _knob("PS_S", 3)
PS_A = _knob("PS_A", 1)
PS_O = _knob("PS_O", 2)
QKV_SPLIT = _knob("QKV_SPLIT", 0)   # 1: alternate qkv evac ACT/DVE
OSB_SPLIT = _knob("OSB_SPLIT", 1)   # 1: alternate osb evac DVE/ACT; 0: all DVE
USE_DIV = _knob("USE_DIV", 0)       # single DVE divide vs reciprocal+mul
REORDER = _knob("REORDER", 2)       # defer last batch's v-proj chunks >= this
KDEFER = _knob("KDEFER", 4)         # defer last batch's k-proj chunks >= this
OSPLIT = _knob("OSPLIT", 2)         # outproj st-groups emitted immediately;
                                    # the rest defer past the next attn chunk


def build_nc():
    nc = bacc.Bacc(None)
    xT = nc.dram_tensor("xT", [BPC, NDC, 128, S], F16, kind="ExternalInput")
    wq = nc.dram_tensor("wq", [NDC, 128, CD], F16, kind="ExternalInput")
    wk = nc.dram_tensor("wk", [NDC, 128, CD], F16, kind="ExternalInput")
    wv = nc.dram_tensor("wv", [NDC, 128, CD], F16, kind="ExternalInput")
    wo = nc.dram_tensor("wo", [HPC, 128, D], F16, kind="ExternalInput")
    msk = nc.dram_tensor("msk", [128, 128], F16, kind="ExternalInput")
    ones = nc.dram_tensor("ones", [128, 128], F16, kind="ExternalInput")
    out = nc.dram_tensor("out", [BPC, S, D], F16, kind="ExternalOutput")

    with tile.TileContext(nc) as tc:
        with (
            tc.tile_pool(name="p_w", bufs=1) as p_w,
            tc.tile_pool(name="p_xs", bufs=XS_BUFS) as p_xs,
            tc.tile_pool(name="p_kv", bufs=2) as p_kv,
            tc.tile_pool(name="p_q", bufs=QT_BUFS) as p_q,
            tc.tile_pool(name="p_at", bufs=AT_BUFS) as p_at,
            tc.tile_pool(name="p_pT", bufs=PT_BUFS) as p_pT,
            tc.tile_pool(name="p_den", bufs=DEN_BUFS) as p_den,
            tc.tile_pool(name="p_rbc", bufs=RBC_BUFS) as p_rbc,
            tc.tile_pool(name="p_osb", bufs=OSB_BUFS) as p_osb,
            tc.tile_pool(name="ps_qkv", bufs=PS_QKV, space="PSUM") as ps_qkv,
            tc.tile_pool(name="ps_s", bufs=PS_S, space="PSUM") as ps_s,
            tc.tile_pool(name="ps_a", bufs=PS_A, space="PSUM") as ps_a,
            tc.tile_pool(name="ps_o", bufs=PS_O, space="PSUM") as ps_o,
        ):
            wq_sb = p_w.tile([128, NDC, CD], F16, tag="wq")
            wk_sb = p_w.tile([128, NDC, CD], F16, tag="wk")
            wv_sb = p_w.tile([128, NDC, CD], F16, tag="wv")
            wo_sb = p_w.tile([128, HPC, D], F16, tag="wo")
            msk_sb = p_w.tile([128, 128], F16, tag="msk")
            ones_sb = p_w.tile([128, 128], F16, tag="ones")
            # DMA transfers drain in issue order; the first q-proj group
            # needs wq[dc] + x0[dc] pairwise, so interleave half-loads of
            # each (PE starts after ~1/4 of the bytes instead of all of
            # them); wk/wv follow (k/v groups run after q's), wo/msk last
            wqr = wq.rearrange("dc dp o -> dp dc o")

            def emit_kproj(sc, xs, kT):
                for h in range(HPC):
                    ps = ps_qkv.tile([128, SC], F32, tag="qkv")
                    for dc in range(NDC):
                        nc.tensor.matmul(
                            ps,
                            wk_sb[:, dc, h * DK:(h + 1) * DK],
                            xs[:, dc, :],
                            start=(dc == 0), stop=(dc == NDC - 1),
                        )
                    nc.scalar.copy(kT[:, h, sc * SC:(sc + 1) * SC], ps)

            def emit_vproj(sc, xs, v_sb):
                for st in range(NST):
                    psv = ps_qkv.tile([128, CD], F32, tag="qkv")
                    for dc in range(NDC):
                        nc.tensor.matmul(
                            psv,
                            xs[:, dc, st * 128:(st + 1) * 128],
                            wv_sb[:, dc, :],
                            start=(dc == 0), stop=(dc == NDC - 1),
                        )
                    nc.scalar.copy(v_sb[:, sc * NST + st, :], psv)

            def emit_proj(b, defer_v_from=NSC, defer_k_from=NSC):
                kT = p_kv.tile([128, HPC, S], F16, tag="kT")
                v_sb = p_kv.tile([128, NSC * NST, CD], F16, tag="v")
                qTs = []
                deferred = []
                deferred_k = []
                for sc in range(NSC):
                    xs = p_xs.tile([128, NDC, SC], F16, tag="xs")
                    xr = xT[b].rearrange("dc dp s -> dp dc s")[
                        :, :, sc * SC:(sc + 1) * SC]
                    if b == 0 and sc == 0:
                        # startup DMAs: wq/x0 halves pairwise on the HWDGE
                        # FIFO (q groups consume them in this order), other
                        # weights on the gpsimd queue
                        half = NDC // 2
                        nc.sync.dma_start(out=wq_sb[:, :half],
                                          in_=wqr[:, :half])
                        nc.sync.dma_start(out=xs[:, :half], in_=xr[:, :half])
                        nc.sync.dma_start(out=wq_sb[:, half:],
                                          in_=wqr[:, half:])
                        nc.sync.dma_start(out=xs[:, half:], in_=xr[:, half:])
                        nc.gpsimd.dma_start(
                            out=wk_sb, in_=wk.rearrange("dc dp o -> dp dc o"))
                        nc.gpsimd.dma_start(
                            out=wv_sb, in_=wv.rearrange("dc dp o -> dp dc o"))
                        nc.gpsimd.dma_start(out=msk_sb, in_=msk[:])
                        nc.gpsimd.dma_start(out=ones_sb, in_=ones[:])
                    else:
                        nc.sync.dma_start(out=xs, in_=xr)
                        if b == 0 and sc == 1:
                            nc.gpsimd.dma_start(
                                out=wo_sb,
                                in_=wo.rearrange("cc cp o -> cp cc o"))
                    qT = p_q.tile([128, HPC, SC], F16, tag="qT")
                    qTs.append(qT)
                    do_k = sc < defer_k_from
                    for h in range(HPC):
                        srcs = ((wq_sb, True), (wk_sb, False)) if do_k \
                            else ((wq_sb, True),)
                        for w_sb, is_q in srcs:
                            ps = ps_qkv.tile([128, SC], F32, tag="qkv")
                            for dc in range(NDC):
                                nc.tensor.matmul(
                                    ps,
                                    w_sb[:, dc, h * DK:(h + 1) * DK],
                                    xs[:, dc, :],
                                    start=(dc == 0), stop=(dc == NDC - 1),
                                )
                            dst = qT[:, h, :] if is_q else \
                                kT[:, h, sc * SC:(sc + 1) * SC]
                            if QKV_SPLIT and not is_q:
                                nc.vector.tensor_copy(dst, ps)
                            else:
                                nc.scalar.copy(dst, ps)
                    if not do_k:
                        deferred_k.append((sc, xs))
                    if sc < defer_v_from:
                        emit_vproj(sc, xs, v_sb)
                    else:
                        deferred.append((sc, xs))
                return kT, v_sb, qTs, deferred, deferred_k

            def emit_attn(b, c, kT, v_sb, qTs):
                attn_c = p_at.tile([128, HPC, SC], F16, tag="attn")
                if True:
                    for h in range(HPC):
                        attps = ps_a.tile([128, SC], F32, tag="attps")
                        den = p_den.tile([128, SC], F16, tag="den")
                        nkt = 4 * c + 4
                        for kt in range(nkt):
                            j = kt - 4 * c  # >=0 on the diagonal block
                            q0 = 0 if j < 0 else j * 128
                            sps = ps_s.tile([128, SC], F32, tag="sps")
                            nc.tensor.matmul(
                                sps[:, q0:],
                                kT[:, h, kt * 128:(kt + 1) * 128],
                                qTs[c][:, h, q0:],
                                start=True, stop=True,
                            )
                            pT = p_pT.tile([128, SC], F16, tag="pT")
                            nc.scalar.activation(
                                pT[:, q0:], sps[:, q0:],
                                mybir.ActivationFunctionType.Exp,
                                scale=SCALE)
                            if j >= 0:
                                nc.vector.tensor_mul(
                                    pT[:, q0:q0 + 128],
                                    pT[:, q0:q0 + 128], msk_sb)
                            nc.tensor.matmul(
                                attps[:, q0:],
                                v_sb[:, kt, h * DK:(h + 1) * DK],
                                pT[:, q0:],
                                start=(kt == 0), stop=(kt == nkt - 1),
                                skip_group_check=True,
                            )
                            if kt == 0:
                                nc.vector.tensor_copy(den, pT)
                            else:
                                nc.vector.tensor_add(
                                    den[:, q0:], den[:, q0:], pT[:, q0:])
                        bc = ps_o.tile([128, SC], F32, tag="ops")
                        nc.tensor.matmul(bc, ones_sb, den,
                                         start=True, stop=True)
                        if USE_DIV:
                            nc.vector.tensor_tensor(
                                attn_c[:, h, :], attps, bc,
                                AluOpType.divide)
                        else:
                            rbc = p_rbc.tile([128, SC], F32, tag="rbc")
                            nc.vector.reciprocal(rbc, bc)
                            nc.vector.tensor_mul(attn_c[:, h, :], attps, rbc)
                return attn_c

            def emit_outproj(b, c, attn_c, sts):
                if True:
                    for st in sts:
                        osb = p_osb.tile([128, D], F16, tag="osb")
                        for oc in range(NSC):
                            ops = ps_o.tile([128, SC], F32, tag="ops")
                            for cc in range(HPC):
                                nc.tensor.matmul(
                                    ops,
                                    attn_c[:, cc, st * 128:(st + 1) * 128],
                                    wo_sb[:, cc, oc * SC:(oc + 1) * SC],
                                    start=(cc == 0), stop=(cc == HPC - 1),
                                )
                            # split PSUM evacuation across DVE and ACT so
                            # neither serializes the outproj filler stream
                            if OSB_SPLIT == 0 or oc % 2 == 0:
                                nc.vector.tensor_copy(
                                    osb[:, oc * SC:(oc + 1) * SC], ops)
                            else:
                                nc.scalar.copy(
                                    osb[:, oc * SC:(oc + 1) * SC], ops)
                        nc.sync.dma_start(
                            out=out[b,
                                    (c * NST + st) * 128:
                                    (c * NST + st + 1) * 128, :],
                            in_=osb,
                        )

            for b in range(BPC):
                # for the last batch, defer the v-projection of the late
                # chunks into its attention phase: attention chunk c only
                # reads v chunks <= c, and these matmuls are the only PE
                # filler available once all other projections are done
                last = b == BPC - 1
                dv = REORDER if last else NSC
                dk = KDEFER if last else NSC
                kT, v_sb, qTs, deferred, deferred_k = emit_proj(
                    b, defer_v_from=dv, defer_k_from=dk)
                prev = None
                for c in range(NSC):
                    if deferred_k and deferred_k[0][0] <= c:
                        sc_, xs_ = deferred_k.pop(0)
                        emit_kproj(sc_, xs_, kT)
                    a = emit_attn(b, c, kT, v_sb, qTs)
                    if deferred and deferred[0][0] <= c + 1:
                        sc_, xs_ = deferred.pop(0)
                        emit_vproj(sc_, xs_, v_sb)
                    if prev is not None and OSPLIT < NST:
                        emit_outproj(b, c - 1, prev, range(OSPLIT, NST))
                    emit_outproj(b, c, a, range(0, OSPLIT))
                    prev = a
                if OSPLIT < NST:
                    emit_outproj(b, NSC - 1, prev, range(OSPLIT, NST))
    nc.compile()
    return nc


_NC_CACHE = None


def _prep_in_maps(x, Wq, Wk, Wv, Wo):
    x = np.asarray(x, dtype=np.float32)
    xTr = np.ascontiguousarray(x.transpose(0, 2, 1)).reshape(
        B, NDC, 128, S).astype(np.float16)

    kk = np.arange(128)[:, None]
    qq = np.arange(128)[None, :]
    msk = (kk <= qq).astype(np.float16)
    ones = np.ones((128, 128), dtype=np.float16)

    in_maps = []
    for core in range(NCORES):
        bg, hg = divmod(core, HPC)
        r0, r1 = hg * CD, (hg + 1) * CD
        in_maps.append({
            "xT": xTr[2 * bg:2 * bg + 2],
            "wq": np.ascontiguousarray(
                np.asarray(Wq, np.float32)[r0:r1].T).reshape(
                    NDC, 128, CD).astype(np.float16),
            "wk": np.ascontiguousarray(
                np.asarray(Wk, np.float32)[r0:r1].T).reshape(
                    NDC, 128, CD).astype(np.float16),
            "wv": np.ascontiguousarray(
                np.asarray(Wv, np.float32)[r0:r1].T).reshape(
                    NDC, 128, CD).astype(np.float16),
            "wo": np.ascontiguousarray(
                np.asarray(Wo, np.float32)[:, r0:r1].T).reshape(
                    HPC, 128, D).astype(np.float16),
            "msk": msk,
            "ones": ones,
        })
    return in_maps


def kernel(x, Wq, Wk, Wv, Wo):
    global _NC_CACHE
    in_maps = _prep_in_maps(x, Wq, Wk, Wv, Wo)
    if _NC_CACHE is None:
        _NC_CACHE = build_nc()
    res = run_bass_kernel_spmd(_NC_CACHE, in_maps, list(range(NCORES)))
    total = np.zeros((B, S, D), dtype=np.float32)
    for core in range(NCORES):
        bg = core // HPC
        total[2 * bg:2 * bg + 2] += res.results[core]["out"].astype(np.float32)
    return total


# revision 4
# speedup vs baseline: 1.0012x; 1.0012x over previous
"""Causal MHA kernel for TRN2 — v2.

Sharding: 8 cores = 2 batch-groups x 4 head-groups. Core (bg, hg) computes
batches {2bg, 2bg+1} for heads {4hg..4hg+3}: q/k/v projection, causal
attention, and the output-projection partial product over its 512 input
channels. Host sums the 4 head-group partials per batch (the tensor-parallel
all-reduce), 4x less output traffic + host work than 8-way head sharding.

Datapath is fp16 end-to-end (PE runs fp16 at 1 cycle/row with no free-dim
constraint, DVE gets 4x on fp16 SBUF operands, DMA bytes halve); PSUM stays
fp32. For N(0,1)-scaled data fp16 (10 mantissa bits) is ~8x more precise
than bf16.

Attention is triangle-trimmed: for the diagonal 512x512 block, k-tile j only
computes/exps/accumulates the q-range [128j, 512), and a single 128x128
lower-tri mask handles the partial tile. Softmax denominator: DVE
accumulates exp tiles (fp16, 4x mode), one all-ones matmul per (c,h)
reduces across partitions and broadcasts.
"""
import numpy as np
import ml_dtypes

import concourse.bass as bass
import concourse.mybir as mybir
from concourse.alu_op_type import AluOpType
import concourse.tile as tile
from concourse import bacc
from concourse.bass_utils import run_bass_kernel_spmd

B, S, D = 4, 2048, 2048
H, DK = 16, 128
NCORES = 8
BPC = 2                    # batches per core
HPC = 4                    # heads per core
CD = HPC * DK              # 512 out-channels per core per projection
SC = 512                   # q/score chunk
NSC = S // SC              # 4
NST = SC // 128            # 4 s-tiles per chunk
NDC = D // 128             # 16 d-chunks
F16 = mybir.dt.float16
F32 = mybir.dt.float32
SCALE = 1.0 / np.sqrt(DK)

import os as _os
def _knob(name, default):
    return int(_os.environ.get(name, str(default)))
XS_BUFS = _knob("XS_BUFS", 2)
QT_BUFS = _knob("QT_BUFS", 4)
AT_BUFS = _knob("AT_BUFS", 3)
PT_BUFS = _knob("PT_BUFS", 6)
DEN_BUFS = _knob("DEN_BUFS", 1)
RBC_BUFS = _knob("RBC_BUFS", 1)
OSB_BUFS = _knob("OSB_BUFS", 2)
PS_QKV = _knob("PS_QKV", 2)
PS_S = _knob("PS_S", 3)
PS_A = _knob("PS_A", 1)
PS_O = _knob("PS_O", 2)
QKV_SPLIT = _knob("QKV_SPLIT", 0)   # 1: alternate qkv evac ACT/DVE
OSB_SPLIT = _knob("OSB_SPLIT", 1)   # 1: alternate osb evac DVE/ACT; 0: all DVE
USE_DIV = _knob("USE_DIV", 0)       # single DVE divide vs reciprocal+mul
WARMUP = _knob("WARMUP", 0)         # dummy PE matmuls to absorb clock ramp
HINTER = _knob("HINTER", 1)         # interleave outproj st-groups per head
REORDER = _knob("REORDER", 2)       # defer last batch's v-proj chunks >= this
KDEFER = _knob("KDEFER", 4)         # defer last batch's k-proj chunks >= this
OSPLIT = _knob("OSPLIT", 2)         # outproj st-groups emitted immediately;
                                    # the rest defer past the next attn chunk


def build_nc():
    nc = bacc.Bacc(None)
    xT = nc.dram_tensor("xT", [BPC, NDC, 128, S], F16, kind="ExternalInput")
    wq = nc.dram_tensor("wq", [NDC, 128, CD], F16, kind="ExternalInput")
    wk = nc.dram_tensor("wk", [NDC, 128, CD], F16, kind="ExternalInput")
    wv = nc.dram_tensor("wv", [NDC, 128, CD], F16, kind="ExternalInput")
    wo = nc.dram_tensor("wo", [HPC, 128, D], F16, kind="ExternalInput")
    msk = nc.dram_tensor("msk", [128, 128], F16, kind="ExternalInput")
    ones = nc.dram_tensor("ones", [128, 128], F16, kind="ExternalInput")
    out = nc.dram_tensor("out", [BPC, S, D], F16, kind="ExternalOutput")

    with tile.TileContext(nc) as tc:
        with (
            tc.tile_pool(name="p_w", bufs=1) as p_w,
            tc.tile_pool(name="p_xs", bufs=XS_BUFS) as p_xs,
            tc.tile_pool(name="p_kv", bufs=2) as p_kv,
            tc.tile_pool(name="p_q", bufs=QT_BUFS) as p_q,
            tc.tile_pool(name="p_at", bufs=AT_BUFS) as p_at,
            tc.tile_pool(name="p_pT", bufs=PT_BUFS) as p_pT,
            tc.tile_pool(name="p_den", bufs=DEN_BUFS) as p_den,
            tc.tile_pool(name="p_rbc", bufs=RBC_BUFS) as p_rbc,
            tc.tile_pool(name="p_osb", bufs=OSB_BUFS) as p_osb,
            tc.tile_pool(name="ps_qkv", bufs=PS_QKV, space="PSUM") as ps_qkv,
            tc.tile_pool(name="ps_s", bufs=PS_S, space="PSUM") as ps_s,
            tc.tile_pool(name="ps_a", bufs=PS_A, space="PSUM") as ps_a,
            tc.tile_pool(name="ps_o", bufs=PS_O, space="PSUM") as ps_o,
        ):
            if WARMUP:
                # dummy matmuls on junk data during the initial DMA wait:
                # PE is otherwise idle ~8us and the clock ramp (full speed
                # only after ~3us of continuous work) would hit the first
                # real matmul groups instead. Output goes to a PSUM slot
                # that is never read.
                junk = p_w.tile([128, 128], F16, tag="junk")
                nc.vector.memset(junk, 0.0)
                jps = ps_o.tile([128, SC], F32, tag="ops")
                for _ in range(WARMUP):
                    nc.tensor.matmul(jps[:, :128], junk, junk,
                                     start=True, stop=True)

            wq_sb = p_w.tile([128, NDC, CD], F16, tag="wq")
            wk_sb = p_w.tile([128, NDC, CD], F16, tag="wk")
            wv_sb = p_w.tile([128, NDC, CD], F16, tag="wv")
            wo_sb = p_w.tile([128, HPC, D], F16, tag="wo")
            msk_sb = p_w.tile([128, 128], F16, tag="msk")
            ones_sb = p_w.tile([128, 128], F16, tag="ones")
            # DMA transfers drain in issue order; the first q-proj group
            # needs wq[dc] + x0[dc] pairwise, so interleave half-loads of
            # each (PE starts after ~1/4 of the bytes instead of all of
            # them); wk/wv follow (k/v groups run after q's), wo/msk last
            wqr = wq.rearrange("dc dp o -> dp dc o")

            def emit_kproj(sc, xs, kT):
                for h in range(HPC):
                    ps = ps_qkv.tile([128, SC], F32, tag="qkv")
                    for dc in range(NDC):
                        nc.tensor.matmul(
                            ps,
                            wk_sb[:, dc, h * DK:(h + 1) * DK],
                            xs[:, dc, :],
                            start=(dc == 0), stop=(dc == NDC - 1),
                        )
                    nc.scalar.copy(kT[:, h, sc * SC:(sc + 1) * SC], ps)

            def emit_vproj(sc, xs, v_sb):
                for st in range(NST):
                    psv = ps_qkv.tile([128, CD], F32, tag="qkv")
                    for dc in range(NDC):
                        nc.tensor.matmul(
                            psv,
                            xs[:, dc, st * 128:(st + 1) * 128],
                            wv_sb[:, dc, :],
                            start=(dc == 0), stop=(dc == NDC - 1),
                        )
                    nc.scalar.copy(v_sb[:, sc * NST + st, :], psv)

            def emit_proj(b, defer_v_from=NSC, defer_k_from=NSC):
                kT = p_kv.tile([128, HPC, S], F16, tag="kT")
                v_sb = p_kv.tile([128, NSC * NST, CD], F16, tag="v")
                qTs = []
                deferred = []
                deferred_k = []
                for sc in range(NSC):
                    xs = p_xs.tile([128, NDC, SC], F16, tag="xs")
                    xr = xT[b].rearrange("dc dp s -> dp dc s")[
                        :, :, sc * SC:(sc + 1) * SC]
                    if b == 0 and sc == 0:
                        # startup DMAs: wq/x0 halves pairwise on the HWDGE
                        # FIFO (q groups consume them in this order), other
                        # weights on the gpsimd queue
                        half = NDC // 2
                        nc.sync.dma_start(out=wq_sb[:, :half],
                                          in_=wqr[:, :half])
                        nc.sync.dma_start(out=xs[:, :half], in_=xr[:, :half])
                        nc.sync.dma_start(out=wq_sb[:, half:],
                                          in_=wqr[:, half:])
                        nc.sync.dma_start(out=xs[:, half:], in_=xr[:, half:])
                        nc.gpsimd.dma_start(
                            out=wk_sb, in_=wk.rearrange("dc dp o -> dp dc o"))
                        nc.gpsimd.dma_start(
                            out=wv_sb, in_=wv.rearrange("dc dp o -> dp dc o"))
                        nc.gpsimd.dma_start(out=msk_sb, in_=msk[:])
                        nc.gpsimd.dma_start(out=ones_sb, in_=ones[:])
                    else:
                        nc.sync.dma_start(out=xs, in_=xr)
                        if b == 0 and sc == 1:
                            nc.gpsimd.dma_start(
                                out=wo_sb,
                                in_=wo.rearrange("cc cp o -> cp cc o"))
                    qT = p_q.tile([128, HPC, SC], F16, tag="qT")
                    qTs.append(qT)
                    do_k = sc < defer_k_from
                    for h in range(HPC):
                        srcs = ((wq_sb, True), (wk_sb, False)) if do_k \
                            else ((wq_sb, True),)
                        for w_sb, is_q in srcs:
                            ps = ps_qkv.tile([128, SC], F32, tag="qkv")
                            for dc in range(NDC):
                                nc.tensor.matmul(
                                    ps,
                                    w_sb[:, dc, h * DK:(h + 1) * DK],
                                    xs[:, dc, :],
                                    start=(dc == 0), stop=(dc == NDC - 1),
                                )
                            dst = qT[:, h, :] if is_q else \
                                kT[:, h, sc * SC:(sc + 1) * SC]
                            if QKV_SPLIT and not is_q:
                                nc.vector.tensor_copy(dst, ps)
                            else:
                                nc.scalar.copy(dst, ps)
                    if not do_k:
                        deferred_k.append((sc, xs))
                    if sc < defer_v_from:
                        emit_vproj(sc, xs, v_sb)
                    else:
                        deferred.append((sc, xs))
                return kT, v_sb, qTs, deferred, deferred_k

            def emit_attn(b, c, kT, v_sb, qTs, between=None):
                attn_c = p_at.tile([128, HPC, SC], F16, tag="attn")
                if True:
                    for h in range(HPC):
                        if between is not None:
                            between(h)
                        attps = ps_a.tile([128, SC], F32, tag="attps")
                        den = p_den.tile([128, SC], F16, tag="den")
                        nkt = 4 * c + 4
                        for kt in range(nkt):
                            j = kt - 4 * c  # >=0 on the diagonal block
                            q0 = 0 if j < 0 else j * 128
                            sps = ps_s.tile([128, SC], F32, tag="sps")
                            nc.tensor.matmul(
                                sps[:, q0:],
                                kT[:, h, kt * 128:(kt + 1) * 128],
                                qTs[c][:, h, q0:],
                                start=True, stop=True,
                            )
                            pT = p_pT.tile([128, SC], F16, tag="pT")
                            nc.scalar.activation(
                                pT[:, q0:], sps[:, q0:],
                                mybir.ActivationFunctionType.Exp,
                                scale=SCALE)
                            if j >= 0:
                                nc.vector.tensor_mul(
                                    pT[:, q0:q0 + 128],
                                    pT[:, q0:q0 + 128], msk_sb)
                            nc.tensor.matmul(
                                attps[:, q0:],
                                v_sb[:, kt, h * DK:(h + 1) * DK],
                                pT[:, q0:],
                                start=(kt == 0), stop=(kt == nkt - 1),
                                skip_group_check=True,
                            )
                            if kt == 0:
                                nc.vector.tensor_copy(den, pT)
                            else:
                                nc.vector.tensor_add(
                                    den[:, q0:], den[:, q0:], pT[:, q0:])
                        bc = ps_o.tile([128, SC], F32, tag="ops")
                        nc.tensor.matmul(bc, ones_sb, den,
                                         start=True, stop=True)
                        if USE_DIV:
                            nc.vector.tensor_tensor(
                                attn_c[:, h, :], attps, bc,
                                AluOpType.divide)
                        else:
                            rbc = p_rbc.tile([128, SC], F32, tag="rbc")
                            nc.vector.reciprocal(rbc, bc)
                            nc.vector.tensor_mul(attn_c[:, h, :], attps, rbc)
                return attn_c

            def emit_outproj(b, c, attn_c, sts):
                if True:
                    for st in sts:
                        osb = p_osb.tile([128, D], F16, tag="osb")
                        for oc in range(NSC):
                            ops = ps_o.tile([128, SC], F32, tag="ops")
                            for cc in range(HPC):
                                nc.tensor.matmul(
                                    ops,
                                    attn_c[:, cc, st * 128:(st + 1) * 128],
                                    wo_sb[:, cc, oc * SC:(oc + 1) * SC],
                                    start=(cc == 0), stop=(cc == HPC - 1),
                                )
                            # split PSUM evacuation across DVE and ACT so
                            # neither serializes the outproj filler stream
                            if OSB_SPLIT == 0 or oc % 2 == 0:
                                nc.vector.tensor_copy(
                                    osb[:, oc * SC:(oc + 1) * SC], ops)
                            else:
                                nc.scalar.copy(
                                    osb[:, oc * SC:(oc + 1) * SC], ops)
                        nc.sync.dma_start(
                            out=out[b,
                                    (c * NST + st) * 128:
                                    (c * NST + st + 1) * 128, :],
                            in_=osb,
                        )

            for b in range(BPC):
                # for the last batch, defer the v-projection of the late
                # chunks into its attention phase: attention chunk c only
                # reads v chunks <= c, and these matmuls are the only PE
                # filler available once all other projections are done
                last = b == BPC - 1
                dv = REORDER if last else NSC
                dk = KDEFER if last else NSC
                kT, v_sb, qTs, deferred, deferred_k = emit_proj(
                    b, defer_v_from=dv, defer_k_from=dk)
                if HINTER:
                    # emit one deferred outproj st-group between attention
                    # heads — filler placed right where the exp-paced
                    # stream leaves PE slack
                    ops_q = []

                    def between(h):
                        if ops_q:
                            c_, a_, st_ = ops_q.pop(0)
                            emit_outproj(b, c_, a_, [st_])

                    for c in range(NSC):
                        if deferred_k and deferred_k[0][0] <= c:
                            sc_, xs_ = deferred_k.pop(0)
                            emit_kproj(sc_, xs_, kT)
                        a = emit_attn(b, c, kT, v_sb, qTs, between)
                        if deferred and deferred[0][0] <= c + 1:
                            sc_, xs_ = deferred.pop(0)
                            emit_vproj(sc_, xs_, v_sb)
                        for st in range(NST):
                            ops_q.append((c, a, st))
                    for c_, a_, st_ in ops_q:
                        emit_outproj(b, c_, a_, [st_])
                else:
                    prev = None
                    for c in range(NSC):
                        if deferred_k and deferred_k[0][0] <= c:
                            sc_, xs_ = deferred_k.pop(0)
                            emit_kproj(sc_, xs_, kT)
                        a = emit_attn(b, c, kT, v_sb, qTs)
                        if deferred and deferred[0][0] <= c + 1:
                            sc_, xs_ = deferred.pop(0)
                            emit_vproj(sc_, xs_, v_sb)
                        if prev is not None and OSPLIT < NST:
                            emit_outproj(b, c - 1, prev, range(OSPLIT, NST))
                        emit_outproj(b, c, a, range(0, OSPLIT))
                        prev = a
                    if OSPLIT < NST:
                        emit_outproj(b, NSC - 1, prev, range(OSPLIT, NST))
    nc.compile()
    return nc


_NC_CACHE = None


def _prep_in_maps(x, Wq, Wk, Wv, Wo):
    x = np.asarray(x, dtype=np.float32)
    xTr = np.ascontiguousarray(x.transpose(0, 2, 1)).reshape(
        B, NDC, 128, S).astype(np.float16)

    kk = np.arange(128)[:, None]
    qq = np.arange(128)[None, :]
    msk = (kk <= qq).astype(np.float16)
    ones = np.ones((128, 128), dtype=np.float16)

    in_maps = []
    for core in range(NCORES):
        bg, hg = divmod(core, HPC)
        r0, r1 = hg * CD, (hg + 1) * CD
        in_maps.append({
            "xT": xTr[2 * bg:2 * bg + 2],
            "wq": np.ascontiguousarray(
                np.asarray(Wq, np.float32)[r0:r1].T).reshape(
                    NDC, 128, CD).astype(np.float16),
            "wk": np.ascontiguousarray(
                np.asarray(Wk, np.float32)[r0:r1].T).reshape(
                    NDC, 128, CD).astype(np.float16),
            "wv": np.ascontiguousarray(
                np.asarray(Wv, np.float32)[r0:r1].T).reshape(
                    NDC, 128, CD).astype(np.float16),
            "wo": np.ascontiguousarray(
                np.asarray(Wo, np.float32)[:, r0:r1].T).reshape(
                    HPC, 128, D).astype(np.float16),
            "msk": msk,
            "ones": ones,
        })
    return in_maps


def kernel(x, Wq, Wk, Wv, Wo):
    global _NC_CACHE
    in_maps = _prep_in_maps(x, Wq, Wk, Wv, Wo)
    if _NC_CACHE is None:
        _NC_CACHE = build_nc()
    res = run_bass_kernel_spmd(_NC_CACHE, in_maps, list(range(NCORES)))
    total = np.zeros((B, S, D), dtype=np.float32)
    for core in range(NCORES):
        bg = core // HPC
        total[2 * bg:2 * bg + 2] += res.results[core]["out"].astype(np.float32)
    return total
